# revision 17
# baseline (speedup 1.0000x reference)
"""Trainium2 Bass kernel for nn_HNN_layer (dense_mlp, memory regime).

Math: the reference never increments start_i, so every block reads
x[:, 0:fn] with fn <= 13.  The whole module collapses to

    out = sigmoid(relu(x[:, :13] @ W + b) @ fk + fb)          (B, 1)

with W a (13, 22) matrix packed from `kernels`, b = biases,
fk = final_kernel, fb = final_bias.

Device strategy (pure data parallel over 8 cores, Bc = 131072 rows/core,
padded to 52 bursts x 5 groups x 512 columns, fp8 DoubleRow inputs):

  - MM1 row-pairing: bursts run in PAIRS.  The even burst's fp8 DR data
    sits at SBUF partitions 0-34 (PE row strips 0-1), the odd burst's at
    partitions 64-98 (strips 2-3).  Two matmuls with disjoint row-group
    masks stream CONCURRENTLY through the PE array (HW per-subarray
    concurrency), writing the two 512-col halves of one (110, 1024)
    PSUM tile -> MM1 costs ~512 cycles per 2 bursts.
  - h-exit: relu fused into the PSUM->SBUF move (bf16 out, 1024 cols),
    alternating ScalarE / VectorE to balance the two PSUM-capable
    engines (these exits are the kernel's roofline: ~1 col/cycle/engine).
  - MM2: 4 col-tiled matmuls per 4-burst quad run concurrently
    (tile_position=(0,32c)).  Weight VARIANTS place fk at column offset
    5v so 24 bursts (6 variants x 4 strips) pack into ONE z PSUM bank
    via interleaved accumulation chains -> sigmoid evacuates 24 bursts
    per 512 columns (z-exit cost 6656 -> 1536 cols).
  - sigmoid(z + fb) fused into the z PSUM->SBUF exit; one (128, 512)
    f32 DMA per z-bank (3 total, SWDGE on GpSimd).
  - DMA triggers parallelized at startup: SP=x chunks, ACT=w1+cst,
    GpSimd=w2+outputs.
"""

import sys

if "/opt/trn_rl_repo" not in sys.path:
    sys.path.insert(0, "/opt/trn_rl_repo")

from contextlib import ExitStack

import numpy as np
import ml_dtypes

import concourse.bass as bass
import concourse.bacc as bacc
import concourse.mybir as mybir
import concourse.tile as tile
from concourse.bass_utils import run_bass_kernel_spmd

FEATURE_LIST = [10, 13, 13, 7, 3, 6, 3, 13, 5, 4, 6, 4, 5, 4, 4, 5, 4, 3, 3, 7, 3, 3]
NB = len(FEATURE_LIST)  # 22 blocks
FMAX = 13               # max(FEATURE_LIST): only x[:, :13] is ever read
B_TOTAL = 1048576
N_CORES = 8
BC = B_TOTAL // N_CORES       # 131072 rows per core
G = 5                         # batch groups packed per matmul column
N = 512                       # free-dim columns per burst (one PSUM bank)
NBURST = 52                   # ceil(BC / (G*N)) -> padded
NPAIR = NBURST // 2           # 26 row-paired MM1 slots
Q = NBURST * N                # 26624 padded rows per group
QH = NPAIR * N                # 13312 columns per parity half in xg
KA = FMAX + 1                 # 14: features + ones row (bias folded into W1)
KH = KA // 2                  # 7 DR partition-pairs per group
KP8 = KH * G                  # 35 rhs partitions per burst half
MP = NB * G                   # 110 MM1 output partitions / MM2 contraction
W1SCALE = 16.0                # pow2 prescale keeps fp8 W1 out of subnormals
MPAD = 112                    # fp8 DoubleRow weight pair-stride (16-multiple)
ZPB = 24                      # bursts packed per z PSUM bank (6 variants x 4)
NZB = 3                       # z banks: 24 + 24 + 4 bursts
NVAR = 6                      # w2 column variants
# pairs per input DMA chunk; small first chunks so compute starts early
PCHUNKS = [1, 1, 4, 8, 12]
DVE_FRAC = 0.5                # fraction of h-exits on VectorE

BF16 = mybir.dt.bfloat16
FP8 = mybir.dt.float8e4
F32 = mybir.dt.float32
_BUILD_CACHE = {}


def _dve_exit_flags(n_exits, dve_frac=DVE_FRAC):
    """Evenly interleaved True(=VectorE)/False(=ScalarE) schedule."""
    n_dve = round(n_exits * dve_frac)
    flags = []
    acc = 0.0
    for _ in range(n_exits):
        acc += n_dve / n_exits
        if acc >= 1.0 - 1e-9:
            flags.append(True)
            acc -= 1.0
        else:
            flags.append(False)
    return flags


def build_program():
    """Build the SPMD Bass program (one NeuronCore's view)."""
    nc = bacc.Bacc("TRN2", target_bir_lowering=False, debug=False)

    # x: rows 0-34 = even bursts' DR pairs, rows 64-98 = odd bursts'
    # (rows 35-63 zero padding so one 3D-AP DMA covers both halves)
    xg = nc.dram_tensor("xg", [99, 2 * QH], FP8, kind="ExternalInput").ap()
    # w1: identical DR weight block at rows 0-34 and 64-98 (strips 0-1 / 2-3)
    w1d = nc.dram_tensor("w1d", [99, 2 * MPAD], FP8, kind="ExternalInput").ap()
    # w2: 6 variants of (110, 32); variant v has fk/W1SCALE at col 5v+g
    w2d = nc.dram_tensor("w2d", [MP, 32 * NVAR], BF16, kind="ExternalInput").ap()
    cst = nc.dram_tensor("cst", [128, 1], F32, kind="ExternalInput").ap()
    # out[p, 512k + n] = sigmoid(z-bank k, partition p = 32c + 5v + g)
    outd = nc.dram_tensor("out", [128, NZB * N], F32, kind="ExternalOutput").ap()

    with tile.TileContext(nc) as tc, ExitStack() as ctx:
        const = ctx.enter_context(tc.tile_pool(name="const", bufs=1))
        hps_pool = ctx.enter_context(tc.tile_pool(name="hps", bufs=3, space="PSUM"))
        zps_pool = ctx.enter_context(tc.tile_pool(name="zps", bufs=2, space="PSUM"))
        hsb_pool = ctx.enter_context(tc.tile_pool(name="hsb", bufs=8))
        osb_pool = ctx.enter_context(tc.tile_pool(name="osb", bufs=2))

        # ACT warmup on garbage SBUF: trigger the activation table load
        # before any real data lands; also issue w1+cst DMAs from ACT's
        # own HWDGE ring so SP can start streaming x immediately.
        w1_t = const.tile([99, 2 * MPAD], FP8)
        w2_t = const.tile([MP, 32 * NVAR], BF16)
        cst_t = const.tile([128, 1], F32)
        warm = const.tile([128, 1], F32)
        nc.gpsimd.memset(warm[:], 0.0)
        nc.scalar.dma_start(w1_t[:], w1d[:])
        nc.scalar.dma_start(cst_t[:], cst[:])
        nc.gpsimd.dma_start(w2_t[:], w2d[:])
        nc.scalar.activation(
            warm[:], warm[:], mybir.ActivationFunctionType.Sigmoid
        )
        nc.scalar.activation(
            warm[:], warm[:], mybir.ActivationFunctionType.Relu
        )
        fbv_ap = cst_t[:, 0:1]

        xpools = [
            ctx.enter_context(tc.tile_pool(name=f"x{i}", bufs=1))
            for i in range(len(PCHUNKS))
        ]
        p_starts = [sum(PCHUNKS[:i]) for i in range(len(PCHUNKS))]
        xstate = {"t": None, "s": 0}
        hsb_of = {}   # pair -> (110, 1024) bf16 relu'd h
        zps_of = {}   # z-bank k -> zps tile

        w1e = w1_t[0:KP8].rearrange("p (i m) -> p i m", i=2)[:, :, 0:MP]
        w1o = w1_t[64:64 + KP8].rearrange("p (i m) -> p i m", i=2)[:, :, 0:MP]

        def emit_mm1_pair(s, skip_ldw=False):
            """One row-paired MM1 slot: bursts 2s (strips 0-1) and 2s+1
            (strips 2-3) stream concurrently into the two halves of one
            (110, 1024) PSUM tile.  skip_ldw reuses the weight plane still
            loaded by the previous pair (nothing can schedule between: all
            later PE ops transitively depend on this pair's outputs)."""
            if s in p_starts:
                ci = p_starts.index(s)
                npair = PCHUNKS[ci]
                xt = xpools[ci].tile([99, 2 * npair * N], FP8)
                # one trigger covers both parity halves so a pair's two
                # matmuls share one DMA sem and schedule simultaneously
                nc.sync.dma_start(
                    xt[:].rearrange("p (i w) -> p i w", i=2),
                    xg[:].rearrange("p (i q) -> p i q", i=2)[
                        :, :, s * N:(s + npair) * N],
                )
                xstate["t"], xstate["s"] = xt, s
            sl = s - xstate["s"]
            xv = xstate["t"]
            hps = hps_pool.tile([MP, 2 * N], F32)
            xe = xv[0:KP8].rearrange("p (i w) -> p i w", i=2)[
                :, :, sl * N:(sl + 1) * N]
            xo = xv[64:64 + KP8].rearrange("p (i w) -> p i w", i=2)[
                :, :, sl * N:(sl + 1) * N]
            me = nc.tensor.matmul(
                hps[:, 0:N], w1e, xe, start=True, stop=True,
                perf_mode=mybir.MatmulPerfMode.DoubleRow,
                tile_position=(0, 0),
            )
            mo = nc.tensor.matmul(
                hps[:, N:2 * N], w1o, xo, start=True, stop=True,
                perf_mode=mybir.MatmulPerfMode.DoubleRow,
                tile_position=(64, 0),
            )
            if skip_ldw:
                me.ins.ldweights = False
                mo.ins.ldweights = False
            return hps

        def emit_exit(s, hps):
            # column-split across both PSUM-capable engines: ACT reads bank
            # 0, DVE bank 1 (parallel PSUM access), halves finish together
            # so the following MM2 quad schedules as one concurrent group
            hsb = hsb_pool.tile([MP, 2 * N], BF16)
            nc.scalar.activation(
                hsb[:, 0:N], hps[:, 0:N], mybir.ActivationFunctionType.Relu
            )
            nc.vector.tensor_scalar_max(hsb[:, N:2 * N], hps[:, N:2 * N], 0.0)
            hsb_of[s] = hsb

        def emit_mm2_quad(m):
            """MM2 for bursts 4m..4m+3 (4 col-tiled concurrent matmuls).
            Burst t lands in z-bank k=t//24 at partitions 32c+5v+g
            (j=t-24k, c=j%4, v=j//4) via accumulation chains."""
            for c in range(4):
                t = 4 * m + c
                k = t // ZPB
                j = t - ZPB * k
                v = j // 4
                if k not in zps_of:
                    zps_of[k] = zps_pool.tile([128, N], F32, name="zbank",
                                              tag="zbank")
                zps = zps_of[k]
                last_v = (NVAR - 1) if k < 2 else 0
                nc.tensor.matmul(
                    zps[32 * c:32 * c + 32, :],
                    w2_t[:, 32 * v:32 * v + 32],
                    hsb_of[s_of(t)][:, (t % 2) * N:(t % 2) * N + N],
                    start=(v == 0), stop=(v == last_v),
                    tile_position=(0, 32 * c),
                    skip_group_check=True,
                )
            del hsb_of[2 * m]
            del hsb_of[2 * m + 1]

        def s_of(t):
            return t // 2

        def emit_sigmoid_out(k):
            zps = zps_of.pop(k)
            osb = osb_pool.tile([128, N], F32)
            nc.scalar.activation(
                osb[:], zps[:], mybir.ActivationFunctionType.Sigmoid,
                bias=fbv_ap,
            )
            # last bank goes via SP's HWDGE (idle by then, lower latency)
            eng = nc.sync if k == NZB - 1 else nc.gpsimd
            eng.dma_start(outd[:, k * N:(k + 1) * N], osb[:])

        # Software-pipelined macro loop (macro = 2 pairs = 4 bursts).
        # PE order per macro: MM2 quad of macro m-1 first, then the two
        # MM1 pairs (second pair reuses the w1 weight plane), so exits of
        # macro m-1 run on ACT/DVE under macro m's MM1 slots.
        prev = [emit_mm1_pair(0), emit_mm1_pair(1, skip_ldw=True)]
        emit_exit(0, prev[0])
        emit_exit(1, prev[1])
        NMAC = NBURST // 4  # 13
        for m in range(1, NMAC):
            emit_mm2_quad(m - 1)
            if m - 1 == 5:
                emit_sigmoid_out(0)
            if m - 1 == 11:
                emit_sigmoid_out(1)
            h0 = emit_mm1_pair(2 * m)
            h1 = emit_mm1_pair(2 * m + 1, skip_ldw=True)
            emit_exit(2 * m, h0)
            emit_exit(2 * m + 1, h1)
        emit_mm2_quad(NMAC - 1)
        emit_sigmoid_out(2)

    nc.compile()
    return nc


def _pack_host_inputs(x, kernels, biases, final_kernel):
    """Build per-core device arrays from the full inputs."""
    W = np.zeros((FMAX, NB), np.float32)
    off = 0
    for i, fn in enumerate(FEATURE_LIST):
        W[:fn, i] = np.asarray(kernels[off:off + fn, 0], np.float32)
        off += fn
    b = np.asarray(biases, np.float32)
    fk = np.asarray(final_kernel[:, 0], np.float32)

    # W_aug rows 0..12 = W * W1SCALE, row 13 = b * W1SCALE (ones-row bias)
    Wa = np.zeros((KA, NB), np.float32)
    Wa[:FMAX] = W * W1SCALE
    Wa[FMAX] = b * W1SCALE
    blk = Wa.reshape(KH, 2, NB)  # [p, i, col]: row 2p+i
    w1_half = np.zeros((KP8, 2, MPAD), np.float32)
    for g in range(G):
        w1_half[KH * g:KH * (g + 1), :, NB * g:NB * (g + 1)] = blk
    w1 = np.zeros((99, 2 * MPAD), np.float32)
    w1[0:KP8] = w1_half.reshape(KP8, 2 * MPAD)
    w1[64:64 + KP8] = w1_half.reshape(KP8, 2 * MPAD)
    w1 = w1.astype(ml_dtypes.float8_e4m3)

    # w2 variants: variant v column (5v + g) of slice v holds fk / W1SCALE
    w2 = np.zeros((MP, 32 * NVAR), np.float32)
    for v in range(NVAR):
        for g in range(G):
            w2[NB * g:NB * (g + 1), 32 * v + 5 * v + g] = fk / W1SCALE
    w2 = w2.astype(ml_dtypes.bfloat16)

    cst = np.zeros((128, 1), np.float32)

    x13 = np.ascontiguousarray(np.asarray(x[:, :FMAX], np.float32)).astype(
        ml_dtypes.float8_e4m3
    )
    one = ml_dtypes.float8_e4m3(1.0)

    in_maps = []
    for cidx in range(N_CORES):
        base = cidx * BC
        # xa[r, g, t, n] = x_aug[r, base + g*Q + t*N + n] (zero-padded)
        xa = np.zeros((KA, G * Q), ml_dtypes.float8_e4m3)
        v = min(G * Q, BC)  # only the global tail pads
        xa[:FMAX, :v] = x13[base:base + v, :].T
        xa[FMAX, :v] = one
        xa = xa.reshape(KA, G, NBURST, N)
        # Xh[e, 7g + p, i, s*N + n] = xa[2p+i, g, 2s+e, n]
        Xh = xa.reshape(KH, 2, G, NPAIR, 2, N).transpose(4, 2, 0, 1, 3, 5)
        Xh = np.ascontiguousarray(Xh).reshape(2, KP8, 2 * QH)
        X = np.zeros((99, 2 * QH), ml_dtypes.float8_e4m3)
        X[0:KP8] = Xh[0]
        X[64:64 + KP8] = Xh[1]
        in_maps.append({"xg": X, "w1d": w1, "w2d": w2, "cst": cst})
    return in_maps


def run(x, kernels, biases, final_kernel, final_bias, trace=False, **spmd_kwargs):
    if "nc" not in _BUILD_CACHE:
        _BUILD_CACHE["nc"] = build_program()
    nc = _BUILD_CACHE["nc"]

    fb = float(np.asarray(final_bias).reshape(-1)[0])
    in_maps = _pack_host_inputs(x, kernels, biases, final_kernel)
    for m in in_maps:
        m["cst"][:, 0] = fb
    res = run_bass_kernel_spmd(
        nc, in_maps, list(range(N_CORES)), trace=trace, **spmd_kwargs
    )
    outs = []
    for cidx in range(N_CORES):
        op = np.asarray(res.results[cidx]["out"], np.float32)  # (128, 3*N)
        # op[32c + 5v + g, k*N + n] = y[base + g*Q + (24k + 4v + c)*N + n]
        arr = op.reshape(4, 32, NZB, N)[:, :30].reshape(4, NVAR, G, NZB, N)
        y5 = arr.transpose(2, 3, 1, 0, 4)        # [g, k, v, c, n]
        y5 = y5.reshape(G, NZB * NVAR * 4, N)    # flat t = 24k + 4v + c
        outs.append(y5[:, :NBURST].reshape(-1)[:BC])
    y = np.concatenate(outs).reshape(B_TOTAL, 1)
    return y, res


def kernel(x, kernels, biases, final_kernel, final_bias):
    y, _ = run(x, kernels, biases, final_kernel, final_bias, trace=False)
    return y
